# revision 1
# baseline (speedup 1.0000x reference)
"""Trainium2 Bass kernel for MultiHeadLinearAttentionLayer.

Problem (hardcoded shapes): B=4, L=S=2048, D_MODEL=1024, N_HEADS=16, HEAD_DIM=64.
  q/k/v = x @ W + b; RoPE(q), RoPE(k); qf/kf = elu(.)+1; kf masked by key_lengths;
  kv = kf^T v, ksum = sum kf; out = (qf @ kv) / (qf @ ksum + eps); y = out @ Wo + bo.

Sharding: 8 cores = 4 batches x 2 head-groups (8 heads each). Each core computes a
partial y (its head-group's contribution through Wo rows); host sums the two
partials per batch. All matmuls in bf16 (fp32 PSUM accumulation).

Per-core layout:
  Q path feature-major (dq on partitions): proj -> +bq -> RoPE (rot via PE matmul
    with block rotation matrix R) -> elu+1 -> qf [4][128,2048].
  K/V paths token-major (tokens on partitions): proj -> +b -> RoPE via free-dim
    half-swap (head-dim features permuted [evens|odds] per head via W col perm)
    -> elu+1 -> kf; v'' = [v | 1] * mask -> kv' = kf^T v'' accumulated in PSUM.
  Attention: den = qf . ksum via block matmul; out' = qf @ kv' (token-major);
    O = out' * recip(den+eps); O^T via PE transpose; y = O^T.T @ Wo (+bo/2).
"""

import os
import numpy as np
import ml_dtypes

import concourse.bacc as bacc
import concourse.mybir as mybir
from concourse import tile
from concourse.bass_utils import run_bass_kernel_spmd

BF16 = mybir.dt.bfloat16
F32 = mybir.dt.float32
AF = mybir.ActivationFunctionType
ALU = mybir.AluOpType
BF = ml_dtypes.bfloat16

D_MODEL = 1024
N_HEADS = 16
HEAD_DIM = 64
ROPE_THETA = 10000.0
EPS = 1e-6
T = 2048          # L = S
NT = T // 128     # 16 token tiles
NC_ = 4           # token chunks of 512
NK = D_MODEL // 128   # 8 contraction tiles
DQ = 512          # per-core head dims (8 heads x 64)
NJ = DQ // 128    # 4 dq tiles
NH = 8            # heads per core

LAST_RESULTS = None  # stashed BassKernelResults for test harnesses


def _build_program(with_bq, with_bk, with_bv, with_bo):
    PHASE = int(os.environ.get("KERNEL_PHASE", "4"))
    nc = bacc.Bacc("TRN2", target_bir_lowering=False)

    xq = nc.declare_dram_parameter("xq", [T, D_MODEL], BF16, isOutput=False)
    xk = nc.declare_dram_parameter("xk", [T, D_MODEL], BF16, isOutput=False)
    xv = nc.declare_dram_parameter("xv", [T, D_MODEL], BF16, isOutput=False)
    wq = nc.declare_dram_parameter("wq", [D_MODEL, DQ], BF16, isOutput=False)
    wk = nc.declare_dram_parameter("wk", [D_MODEL, DQ], BF16, isOutput=False)
    wv = nc.declare_dram_parameter("wv", [D_MODEL, DQ], BF16, isOutput=False)
    wo = nc.declare_dram_parameter("wo", [DQ, D_MODEL], BF16, isOutput=False)
    cosfm = nc.declare_dram_parameter("cosfm", [128, T], BF16, isOutput=False)
    sinfm = nc.declare_dram_parameter("sinfm", [128, T], BF16, isOutput=False)
    costm = nc.declare_dram_parameter("costm", [128, NT * DQ], BF16, isOutput=False)
    sintm = nc.declare_dram_parameter("sintm", [128, NT * DQ], BF16, isOutput=False)
    rt = nc.declare_dram_parameter("rt", [128, 128], BF16, isOutput=False)
    ident = nc.declare_dram_parameter("ident", [128, 128], BF16, isOutput=False)
    maskc = nc.declare_dram_parameter("maskc", [128, NT], F32, isOutput=False)
    bq = nc.declare_dram_parameter("bq", [1, DQ], BF16, isOutput=False) if with_bq else None
    bk = nc.declare_dram_parameter("bk", [1, DQ], BF16, isOutput=False) if with_bk else None
    bv = nc.declare_dram_parameter("bv", [1, DQ], BF16, isOutput=False) if with_bv else None
    bo = nc.declare_dram_parameter("bo", [1, D_MODEL], BF16, isOutput=False) if with_bo else None
    y = nc.declare_dram_parameter("y", [T, D_MODEL], F32, isOutput=True)

    with tile.TileContext(nc) as tc:
        with tc.tile_pool(name="sb", bufs=1) as sb, \
             tc.tile_pool(name="ps", bufs=1, space="PSUM") as ps:

            # ---- constant loads ----
            wq_sb = sb.tile([128, NK, DQ], BF16, tag="w", bufs=3)
            nc.sync.dma_start(wq_sb[:], wq.rearrange("(k p) c -> p k c", p=128))
            rt_sb = sb.tile([128, 128], BF16, tag="rt")
            nc.sync.dma_start(rt_sb[:], rt[:])
            id_sb = sb.tile([128, 128], BF16, tag="ident")
            nc.sync.dma_start(id_sb[:], ident[:])
            cosf = sb.tile([128, T], BF16, tag="fm", bufs=2)
            nc.sync.dma_start(cosf[:], cosfm[:])
            sinf = sb.tile([128, T], BF16, tag="fm", bufs=2)
            nc.sync.dma_start(sinf[:], sinfm[:])
            ones = sb.tile([1, 512], BF16, tag="ones")
            nc.vector.memset(ones[:], 1.0)
            zrow = sb.tile([1, 512], BF16, tag="zrow")
            nc.vector.memset(zrow[:], 0.0)
            if with_bq:
                bq_sb = sb.tile([1, DQ], BF16, tag="brow", bufs=4)
                nc.sync.dma_start(bq_sb[:], bq[:])

            # xq transposed tiles (feature-major), then Q phase
            xqt = []
            for k in range(NK):
                t_ = sb.tile([128, T], BF16, tag="xt", bufs=12, name=f"xqt{k}")
                nc.sync.dma_start_transpose(t_[:], xq[:, 128 * k:128 * (k + 1)])
                xqt.append(t_)

            qf = [sb.tile([128, T], BF16, tag="qf", bufs=NJ, name=f"qf{j}")
                  for j in range(NJ)]

            with nc.named_scope("qproj"):
                for j in range(NJ):
                    psqs = []
                    for c in range(NC_):
                        psq = ps.tile([128, 512], F32, tag="mm", bufs=7, name="psq")
                        if with_bq:
                            nc.tensor.matmul(psq[:], bq_sb[:, 128 * j:128 * (j + 1)],
                                             ones[:], start=True, stop=False)
                        psqs.append(psq)
                    for k in range(NK):
                        for c in range(NC_):
                            nc.tensor.matmul(
                                psqs[c][:], wq_sb[:, k, 128 * j:128 * (j + 1)],
                                xqt[k][:, 512 * c:512 * (c + 1)],
                                start=(k == 0 and not with_bq), stop=(k == NK - 1))
                    for c in range(NC_):
                        psq = psqs[c]
                        qt = sb.tile([128, 512], BF16, tag="tmp", bufs=10, name="qt")
                        nc.vector.tensor_copy(qt[:], psq[:])
                        # RoPE: q' = qt*cos + (R qt)*sin
                        rotp = ps.tile([128, 512], F32, tag="mm", bufs=7, name="rotp")
                        nc.tensor.matmul(rotp[:], rt_sb[:], qt[:], start=True, stop=True)
                        t1 = sb.tile([128, 512], BF16, tag="tmp", bufs=10, name="t1")
                        nc.vector.tensor_tensor(
                            t1[:], qt[:], cosf[:, 512 * c:512 * (c + 1)], ALU.mult)
                        t2 = sb.tile([128, 512], BF16, tag="tmp", bufs=10, name="t2")
                        nc.vector.tensor_tensor(
                            t2[:], rotp[:], sinf[:, 512 * c:512 * (c + 1)], ALU.mult)
                        q2 = sb.tile([128, 512], BF16, tag="tmp", bufs=10, name="q2")
                        nc.vector.tensor_tensor(q2[:], t1[:], t2[:], ALU.add)
                        # elu+1 = min(exp,1) + relu
                        qe = sb.tile([128, 512], BF16, tag="tmp", bufs=10, name="qe")
                        nc.scalar.activation(qe[:], q2[:], AF.Exp)
                        qr = sb.tile([128, 512], BF16, tag="tmp", bufs=10, name="qr")
                        nc.scalar.activation(qr[:], q2[:], AF.Relu)
                        nc.vector.scalar_tensor_tensor(
                            qf[j][:, 512 * c:512 * (c + 1)], qe[:], 1.0, qr[:],
                            ALU.min, ALU.add)

            if PHASE == 1:
                dbg = sb.tile([128, 1024], F32, tag="ysb", bufs=3, name="dbg")
                nc.scalar.copy(dbg[:], qf[0][:, 0:1024])
                nc.sync.dma_start(y[0:128, :], dbg[:])
            if PHASE >= 2:
                # ---- K phase (token-major) ----
                wk_sb = sb.tile([128, NK, DQ], BF16, tag="w", bufs=3)
                nc.sync.dma_start(wk_sb[:], wk.rearrange("(k p) c -> p k c", p=128))
                if with_bk:
                    bk_sb = sb.tile([1, DQ], BF16, tag="brow", bufs=4)
                    nc.sync.dma_start(bk_sb[:], bk[:])
                cost = sb.tile([128, NT, DQ], BF16, tag="tm", bufs=2)
                nc.sync.dma_start(cost[:], costm.rearrange("p (m c) -> p m c", m=NT))
                sint = sb.tile([128, NT, DQ], BF16, tag="tm", bufs=2)
                nc.sync.dma_start(sint[:], sintm.rearrange("p (m c) -> p m c", m=NT))
                xkt = []
                for k in range(NK):
                    t_ = sb.tile([128, T], BF16, tag="xt", bufs=12, name=f"xkt{k}")
                    nc.sync.dma_start_transpose(t_[:], xk[:, 128 * k:128 * (k + 1)])
                    xkt.append(t_)

                kf = [sb.tile([128, DQ], BF16, tag="kf", bufs=NT, name=f"kf{m}")
                      for m in range(NT)]

                with nc.named_scope("kproj"):
                    for m in range(NT):
                        psk = ps.tile([128, 512], F32, tag="mm", bufs=7, name="psk")
                        first = True
                        if with_bk:
                            nc.tensor.matmul(psk[:], ones[:, 0:128], bk_sb[:],
                                             start=True, stop=False)
                            first = False
                        for k in range(NK):
                            nc.tensor.matmul(
                                psk[:], xkt[k][:, 128 * m:128 * (m + 1)],
                                wk_sb[:, k, :], start=first, stop=(k == NK - 1))
                            first = False
                        ksb = sb.tile([128, 512], BF16, tag="tmp", bufs=10, name="ksb")
                        nc.scalar.copy(ksb[:], psk[:])
                        # RoPE token-major, [evens|odds] per-head halves
                        kv8 = ksb.rearrange("p (h s i) -> p h s i", h=NH, s=2, i=32)
                        t1 = sb.tile([128, 512], BF16, tag="tmp", bufs=10, name="t1k")
                        nc.vector.tensor_tensor(t1[:], ksb[:], cost[:, m, :], ALU.mult)
                        t2 = sb.tile([128, 512], BF16, tag="tmp", bufs=10, name="t2k")
                        t28 = t2.rearrange("p (h s i) -> p h s i", h=NH, s=2, i=32)
                        sin8 = sint[:, m, :].rearrange("p (h s i) -> p h s i", h=NH, s=2, i=32)
                        nc.vector.tensor_tensor(t28[:, :, 0, :], kv8[:, :, 1, :],
                                                sin8[:, :, 0, :], ALU.mult)
                        nc.vector.tensor_tensor(t28[:, :, 1, :], kv8[:, :, 0, :],
                                                sin8[:, :, 1, :], ALU.mult)
                        k2 = sb.tile([128, 512], BF16, tag="tmp", bufs=10, name="k2")
                        nc.vector.tensor_tensor(k2[:], t1[:], t2[:], ALU.add)
                        ke = sb.tile([128, 512], BF16, tag="tmp", bufs=10, name="ke")
                        nc.scalar.activation(ke[:], k2[:], AF.Exp)
                        kr = sb.tile([128, 512], BF16, tag="tmp", bufs=10, name="kr")
                        nc.scalar.activation(kr[:], k2[:], AF.Relu)
                        nc.vector.scalar_tensor_tensor(kf[m][:], ke[:], 1.0, kr[:],
                                                       ALU.min, ALU.add)

                if PHASE == 2:
                    dbg = sb.tile([128, 1024], F32, tag="ysb", bufs=3, name="dbg")
                    nc.scalar.copy(dbg[:, 0:512], kf[0][:])
                    nc.sync.dma_start(y[0:128, :], dbg[:])

            if PHASE >= 3:
                # ---- V phase + kv accumulation ----
                wv_sb = sb.tile([128, NK, DQ], BF16, tag="w", bufs=3)
                nc.sync.dma_start(wv_sb[:], wv.rearrange("(k p) c -> p k c", p=128))
                if with_bv:
                    bv_sb = sb.tile([1, DQ], BF16, tag="brow", bufs=4)
                    nc.sync.dma_start(bv_sb[:], bv[:])
                mk_sb = sb.tile([128, NT], F32, tag="mask")
                nc.sync.dma_start(mk_sb[:], maskc[:])
                wo_sb = sb.tile([128, NJ, D_MODEL], BF16, tag="wo")
                nc.sync.dma_start(wo_sb[:], wo.rearrange("(k p) c -> p k c", p=128))
                xvt = []
                for k in range(NK):
                    t_ = sb.tile([128, T], BF16, tag="xt", bufs=12, name=f"xvt{k}")
                    nc.sync.dma_start_transpose(t_[:], xv[:, 128 * k:128 * (k + 1)])
                    xvt.append(t_)

                kvp_t = ps.tile([128, 512], F32, tag="kv", bufs=1, name="kvp")
                kvp = kvp_t[:, 0:272]
                # open the kv accumulation group: zero the whole bank so later
                # disjoint-region matmuls (start=False) all depend on this one
                nc.tensor.matmul(kvp[:], zrow[:, 0:128], zrow[:, 0:272],
                                 start=True, stop=False)
                with nc.named_scope("vproj"):
                    for m in range(NT):
                        psv = ps.tile([128, 512], F32, tag="mm", bufs=7, name="psv")
                        first = True
                        if with_bv:
                            nc.tensor.matmul(psv[:], ones[:, 0:128], bv_sb[:],
                                             start=True, stop=False)
                            first = False
                        for k in range(NK):
                            nc.tensor.matmul(
                                psv[:], xvt[k][:, 128 * m:128 * (m + 1)],
                                wv_sb[:, k, :], start=first, stop=(k == NK - 1))
                            first = False
                        v2 = sb.tile([128, NH, 68], BF16, tag="vv", bufs=4, name="v2")
                        nc.vector.tensor_scalar_mul(
                            v2[:, :, 0:64], psv.rearrange("p (h i) -> p h i", h=NH),
                            mk_sb[:, m:m + 1])
                        nc.vector.tensor_copy(
                            v2[:, :, 64:68],
                            mk_sb[:, m:m + 1].rearrange("p (a i) -> p a i", a=1)
                            .broadcast_to([128, NH, 4]))
                        # kv' accumulation: head h -> rows 64*(h%2), cols 65*(h//2).
                        # All inside the bank-wide group opened above.
                        for h in range(NH):
                            r0 = 64 * (h % 2)
                            c0 = 68 * (h // 2)
                            nc.tensor.matmul(
                                kvp[r0:r0 + 64, c0:c0 + 68],
                                kf[m][:, 64 * h:64 * (h + 1)],
                                v2[:, h, :],
                                start=False, stop=False,
                                tile_position=(0, r0))

                # close the kv group with a whole-bank +0 matmul (ordered after
                # every kv matmul via full-region overlap)
                nc.tensor.matmul(kvp[:], zrow[:, 0:128], zrow[:, 0:272],
                                 start=False, stop=True)
                # Per-head zero-padded layout: block h holds kv'_h in its
                # parity's 64 rows, zeros elsewhere, so the out'-stage can
                # contract the full 128 partitions (K=64+M=128 matmuls are
                # rejected by the runtime).
                kv2 = sb.tile([128, NH, 68], BF16, tag="kvsb")
                nc.vector.memset(kv2[:], 0.0)
                kv2v = kv2.rearrange("p (j s) e -> p j s e", s=2)
                kvpv = kvp.rearrange("p (j e) -> p j e", j=4)
                nc.scalar.copy(kv2v[0:64, :, 0, :], kvpv[0:64])
                nc.scalar.copy(kv2v[64:128, :, 1, :], kvpv[64:128])
                kv_sb = kv2.rearrange("p h e -> p (h e)")

                if PHASE == 3:
                    dbg = sb.tile([128, 1024], F32, tag="ysb", bufs=3, name="dbg")
                    nc.scalar.copy(dbg[:, 0:544], kv_sb[:])
                    nc.sync.dma_start(y[0:128, :], dbg[:])

            if PHASE >= 4:  # 5 = attn minus yproj
                # ---- attention + output projection, per token tile ----
                if with_bo:
                    bo_sb = sb.tile([1, D_MODEL], BF16, tag="bo")
                    nc.sync.dma_start(bo_sb[:], bo[:])

                with nc.named_scope("attn"):
                    osbs = []
                    zs = []
                    # pass A: out' matmuls + z + z-scaled O tiles (token-major)
                    for m in range(NT):
                        osb = sb.tile([128, 512], BF16, tag="osb", bufs=NT, name="osb")
                        zt = sb.tile([128, 8], F32, tag="z", bufs=4, name="zt")
                        z = sb.tile([128, 8], F32, tag="z", bufs=4, name="z")
                        for half in range(2):
                            op_t = ps.tile([128, 512], F32, tag="mm", bufs=7, name="op")
                            op = op_t[:, 0:272]
                            nc.tensor.matmul(op[:], zrow[:, 0:128], zrow[:, 0:272],
                                             start=True, stop=False)
                            for jj in range(2):
                                j = 2 * half + jj
                                nc.tensor.matmul(
                                    op[:, 136 * jj:136 * jj + 136],
                                    qf[j][:, 128 * m:128 * (m + 1)],
                                    kv_sb[:, 136 * j:136 * j + 136],
                                    start=False, stop=False)
                            nc.tensor.matmul(op[:], zrow[:, 0:128], zrow[:, 0:272],
                                             start=False, stop=True)
                            nc.vector.tensor_scalar_add(
                                zt[:, 4 * half:4 * half + 4],
                                op.rearrange("p (hh e) -> p hh e", hh=4)[:, :, 64:65],
                                EPS)
                            nc.vector.reciprocal(z[:, 4 * half:4 * half + 4],
                                                 zt[:, 4 * half:4 * half + 4])
                            for hh in range(4):
                                h = 4 * half + hh
                                nc.vector.tensor_scalar_mul(
                                    osb[:, 64 * h:64 * (h + 1)],
                                    op[:, 68 * hh:68 * hh + 64],
                                    z[:, h:h + 1])
                        osbs.append(osb)
                        zs.append(z)

                    # pass B: transpose O, output projection, store
                    for m in range(NT):
                        osb = osbs[m]
                        ot = []
                        for j in range(NJ):
                            otp = ps.tile([128, 128], BF16, tag="mm", bufs=7, name="otp")
                            nc.tensor.transpose(otp[:], osb[:, 128 * j:128 * (j + 1)],
                                                id_sb[:])
                            o_ = sb.tile([128, 128], BF16, tag="ot", bufs=8, name="ot")
                            if j % 2 == 0:
                                nc.scalar.copy(o_[:], otp[:])
                            else:
                                nc.vector.tensor_copy(o_[:], otp[:])
                            ot.append(o_)
                        yps = []
                        for c2 in range(2):
                            yp = ps.tile([128, 512], F32, tag="mm", bufs=7, name="yp")
                            if with_bo:
                                nc.tensor.matmul(yp[:], ones[:, 0:128],
                                                 bo_sb[:, 512 * c2:512 * (c2 + 1)],
                                                 start=True, stop=False)
                            yps.append(yp)
                        for j in range(NJ):
                            for c2 in range(2):
                                nc.tensor.matmul(
                                    yps[c2][:], ot[j][:],
                                    wo_sb[:, j, 512 * c2:512 * (c2 + 1)],
                                    start=(j == 0 and not with_bo), stop=(j == NJ - 1))
                        for c2 in range(2):
                            ysb = sb.tile([128, 512], F32, tag="ysb", bufs=3, name="ysb")
                            nc.scalar.copy(ysb[:], yps[c2][:])
                            nc.sync.dma_start(
                                y[128 * m:128 * (m + 1), 512 * c2:512 * (c2 + 1)], ysb[:])

    nc.compile()
    return nc


def _host_prep(queries, keys, values, key_lengths, Wq, bq, Wk, bk, Wv, bv, Wo, bo):
    """Build the per-core input maps (host side: slicing, dtype cast, tables)."""
    B = queries.shape[0]
    # per-head [evens|odds] feature permutation
    pat = np.concatenate([np.arange(0, HEAD_DIM, 2), np.arange(1, HEAD_DIM, 2)])
    perm = np.concatenate([h * HEAD_DIM + pat for h in range(NH)])  # within 512

    inv_freq = 1.0 / (ROPE_THETA ** (np.arange(0, HEAD_DIM, 2, dtype=np.float64)
                                     / HEAD_DIM))  # [32]
    t = np.arange(T, dtype=np.float64)
    ang = t[:, None] * inv_freq[None, :]           # [T, 32]
    cos32 = np.cos(ang).astype(np.float32)
    sin32 = np.sin(ang).astype(np.float32)

    # feature-major tables [128, T]: row r: block = r % 64; i = block % 32
    idx = np.arange(128) % HEAD_DIM
    fidx = np.where(idx < 32, idx, idx - 32)
    cosfm = cos32[:, fidx].T.astype(BF)            # [128, T]
    sinfm = sin32[:, fidx].T.astype(BF)

    # token-major tables [T, 512] -> [128, NT*512] partition-major
    cidx = np.arange(DQ) % HEAD_DIM
    cf = np.where(cidx < 32, cidx, cidx - 32)
    costm_full = cos32[:, cf]                      # [T, 512]
    ssign = np.where(cidx < 32, -1.0, 1.0).astype(np.float32)
    sintm_full = sin32[:, cf] * ssign
    def tmshape(a):  # [T, 512] -> [128, NT*512]
        return np.ascontiguousarray(
            a.reshape(NT, 128, DQ).transpose(1, 0, 2).reshape(128, NT * DQ)).astype(BF)
    costm = tmshape(costm_full)
    sintm = tmshape(sintm_full)

    # rotation matrix R (permuted layout), lhsT = R.T
    R = np.zeros((128, 128), np.float32)
    for base in (0, 64):
        R[base + 0:base + 32, base + 32:base + 64] = -np.eye(32)
        R[base + 32:base + 64, base + 0:base + 32] = np.eye(32)
    rt = np.ascontiguousarray(R.T).astype(BF)
    ident = np.eye(128, dtype=np.float32).astype(BF)

    with_bq = bool(np.any(np.asarray(bq)))
    with_bk = bool(np.any(np.asarray(bk)))
    with_bv = bool(np.any(np.asarray(bv)))
    with_bo = bool(np.any(np.asarray(bo)))

    xq_b = [np.asarray(queries[b]).astype(BF) for b in range(B)]
    xk_b = [np.asarray(keys[b]).astype(BF) for b in range(B)]
    xv_b = [np.asarray(values[b]).astype(BF) for b in range(B)]
    Wq = np.asarray(Wq, np.float32); Wk = np.asarray(Wk, np.float32)
    Wv = np.asarray(Wv, np.float32); Wo = np.asarray(Wo, np.float32)
    bq = np.asarray(bq, np.float32); bk = np.asarray(bk, np.float32)
    bv = np.asarray(bv, np.float32); bo = np.asarray(bo, np.float32)

    in_maps = []
    for core in range(2 * B):
        b, g = core // 2, core % 2
        sl = slice(DQ * g, DQ * (g + 1))
        mask = (np.arange(T) < int(key_lengths[b])).astype(np.float32)
        maskc = np.ascontiguousarray(mask.reshape(NT, 128).T)
        m = {
            "xq": xq_b[b], "xk": xk_b[b], "xv": xv_b[b],
            "wq": np.ascontiguousarray(Wq[:, sl][:, perm]).astype(BF),
            "wk": np.ascontiguousarray(Wk[:, sl][:, perm]).astype(BF),
            "wv": np.ascontiguousarray(Wv[:, sl]).astype(BF),
            "wo": np.ascontiguousarray(Wo[sl, :]).astype(BF),
            "cosfm": cosfm, "sinfm": sinfm, "costm": costm, "sintm": sintm,
            "rt": rt, "ident": ident, "maskc": maskc,
        }
        if with_bq:
            m["bq"] = bq[sl][perm].reshape(1, DQ).astype(BF)
        if with_bk:
            m["bk"] = bk[sl][perm].reshape(1, DQ).astype(BF)
        if with_bv:
            m["bv"] = bv[sl].reshape(1, DQ).astype(BF)
        if with_bo:
            m["bo"] = (bo / 2.0).reshape(1, D_MODEL).astype(BF)
        in_maps.append(m)
    return in_maps, (with_bq, with_bk, with_bv, with_bo)


def kernel(queries, keys, values, attn_mask, query_lengths, key_lengths,
           Wq, bq, Wk, bk, Wv, bv, Wo, bo):
    global LAST_RESULTS
    B = queries.shape[0]
    in_maps, bias_flags = _host_prep(queries, keys, values, key_lengths,
                                     Wq, bq, Wk, bk, Wv, bv, Wo, bo)
    nc = _build_program(*bias_flags)
    res = run_bass_kernel_spmd(nc, in_maps, core_ids=list(range(2 * B)))
    LAST_RESULTS = res
    out = np.zeros((B, T, D_MODEL), np.float32)
    for b in range(B):
        out[b] = res.results[2 * b]["y"] + res.results[2 * b + 1]["y"]
    return out



# revision 10
# speedup vs baseline: 1.0517x; 1.0517x over previous
"""Trainium2 Bass kernel for MultiHeadLinearAttentionLayer.

Problem (hardcoded shapes): B=4, L=S=2048, D_MODEL=1024, N_HEADS=16, HEAD_DIM=64.
  q/k/v = x @ W + b; RoPE(q), RoPE(k); qf/kf = elu(.)+1; kf masked by key_lengths;
  kv = kf^T v, ksum = sum kf; out = (qf @ kv) / (qf @ ksum + eps); y = out @ Wo + bo.

Sharding: 8 cores = 4 batches x 2 head-groups (8 heads each). Each core computes a
partial y (its head-group's contribution through Wo rows); host sums the two
partials per batch and adds bo. All matmuls bf16 (fp32 PSUM accumulation).

v2 design (vs the DMA-transpose baseline):
  - x^T computed on HOST: xq/xk/xv arrive pre-transposed [D, T] -> plain DMA loads.
  - Q path feature-major: proj -> RoPE (rotate via PE matmul with R) -> elu+1
    -> qf[j] [128 dq, T].
  - K/V token-major: proj -> RoPE via free-dim half-swap (permuted W cols) ->
    elu+1 -> kf[m] [128 tok, 512]; v2 = [v*mask | mask]; kv' accumulated in PSUM.
  - kv' repacked to per-j block-diagonal kvblk [128,128] (+ kspack [128,8] ksums),
    so the whole attention tail is feature-major with N=512 matmuls:
      den stripes = kspack_j^T @ qf_j  (PSUM partitions 32j:32j+2)
      zrec = 1/den (vector reciprocal; eps dropped -- host handles key_len==0)
      zexp_j = esel_j^T @ zrec_j  (PE partition-broadcast), O_j = op_j * zexp_j
      y = sum_j O_j^T.T @ Wo_j   (no PE transposes, no zero-matmuls)
  - Elementwise work split across Vector/Scalar/GpSimd to keep PE the bottleneck.
"""

import os
import numpy as np
import ml_dtypes

import concourse.bacc as bacc
import concourse.mybir as mybir
from concourse import tile
from concourse.bass_utils import run_bass_kernel_spmd

BF16 = mybir.dt.bfloat16
F32 = mybir.dt.float32
AF = mybir.ActivationFunctionType
ALU = mybir.AluOpType
BF = ml_dtypes.bfloat16

D_MODEL = 1024
N_HEADS = 16
HEAD_DIM = 64
ROPE_THETA = 10000.0
T = 2048          # L = S
NT = T // 128     # 16 token tiles
NC_ = 4           # token chunks of 512
NK = D_MODEL // 128   # 8 contraction tiles
DQ = 512          # per-core head dims (8 heads x 64)
NJ = DQ // 128    # 4 dq tiles
NH = 8            # heads per core

LAST_RESULTS = None  # stashed BassKernelResults for test harnesses


def _build_program(with_bq, with_bk, with_bv):
    GPS = int(os.environ.get("KERNEL_GPS", "1"))   # gpsimd offload on/off
    nc = bacc.Bacc("TRN2", target_bir_lowering=False)

    xqt_d = nc.declare_dram_parameter("xqt", [D_MODEL, T], BF16, isOutput=False)
    xkt_d = nc.declare_dram_parameter("xkt", [D_MODEL, T], BF16, isOutput=False)
    xvt_d = nc.declare_dram_parameter("xvt", [D_MODEL, T], BF16, isOutput=False)
    wq = nc.declare_dram_parameter("wq", [D_MODEL, DQ], BF16, isOutput=False)
    wk = nc.declare_dram_parameter("wk", [D_MODEL, DQ], BF16, isOutput=False)
    wv = nc.declare_dram_parameter("wv", [D_MODEL, DQ], BF16, isOutput=False)
    wo = nc.declare_dram_parameter("wo", [DQ, D_MODEL], BF16, isOutput=False)
    cosfm = nc.declare_dram_parameter("cosfm", [128, T], BF16, isOutput=False)
    sinfm = nc.declare_dram_parameter("sinfm", [128, T], BF16, isOutput=False)
    costm = nc.declare_dram_parameter("costm", [128, NT * DQ], BF16, isOutput=False)
    sintm = nc.declare_dram_parameter("sintm", [128, NT * DQ], BF16, isOutput=False)
    rt = nc.declare_dram_parameter("rt", [128, 128], BF16, isOutput=False)
    eselp = nc.declare_dram_parameter("esel", [128, 128], BF16, isOutput=False)
    maskc = nc.declare_dram_parameter("maskc", [128, NT], F32, isOutput=False)
    bq = nc.declare_dram_parameter("bq", [1, DQ], BF16, isOutput=False) if with_bq else None
    bk = nc.declare_dram_parameter("bk", [1, DQ], BF16, isOutput=False) if with_bk else None
    bv = nc.declare_dram_parameter("bv", [1, DQ], BF16, isOutput=False) if with_bv else None
    y = nc.declare_dram_parameter("y", [T, D_MODEL], F32, isOutput=True)

    with tile.TileContext(nc) as tc:
        with tc.tile_pool(name="sb", bufs=1) as sb, \
             tc.tile_pool(name="ps", bufs=1, space="PSUM") as ps:

            # ---- constants / weights / Q inputs ----
            wq_sb = sb.tile([128, NK, DQ], BF16, tag="w", bufs=3)
            nc.sync.dma_start(wq_sb[:], wq.rearrange("(k p) c -> p k c", p=128))
            rt_sb = sb.tile([128, 128], BF16, tag="rt")
            nc.sync.dma_start(rt_sb[:], rt[:])
            cosf = sb.tile([128, T], BF16, tag="fm", bufs=2)
            nc.sync.dma_start(cosf[:], cosfm[:])
            sinf = sb.tile([128, T], BF16, tag="fm", bufs=2)
            nc.sync.dma_start(sinf[:], sinfm[:])
            ones = sb.tile([1, 512], BF16, tag="ones")
            nc.vector.memset(ones[:], 1.0)
            zrow = sb.tile([1, 512], BF16, tag="zrow")
            nc.vector.memset(zrow[:], 0.0)
            if with_bq:
                bq_sb = sb.tile([1, DQ], BF16, tag="brow", bufs=3)
                nc.sync.dma_start(bq_sb[:], bq[:])

            xqt = []
            for k in range(NK):
                t_ = sb.tile([128, T], BF16, tag="xt", bufs=12, name=f"xqt{k}")
                nc.sync.dma_start(t_[:], xqt_d[128 * k:128 * (k + 1), :])
                xqt.append(t_)

            qf = [sb.tile([128, T], BF16, tag="qf", bufs=NJ, name=f"qf{j}")
                  for j in range(NJ)]

            # ---- Q phase (feature-major) ----
            with nc.named_scope("qproj"):
                for j in range(NJ):
                    psqs = []
                    for c in range(NC_):
                        psq = ps.tile([128, 512], F32, tag="mm", bufs=7, name="psq")
                        if with_bq:
                            nc.tensor.matmul(psq[:], bq_sb[:, 128 * j:128 * (j + 1)],
                                             ones[:], start=True, stop=False)
                        psqs.append(psq)
                    for k in range(NK):
                        for c in range(NC_):
                            nc.tensor.matmul(
                                psqs[c][:], wq_sb[:, k, 128 * j:128 * (j + 1)],
                                xqt[k][:, 512 * c:512 * (c + 1)],
                                start=(k == 0 and not with_bq), stop=(k == NK - 1))
                    for c in range(NC_):
                        psq = psqs[c]
                        qt = sb.tile([128, 512], BF16, tag="tmp", bufs=12, name="qt")
                        nc.scalar.copy(qt[:], psq[:])
                        rotp = ps.tile([128, 512], F32, tag="mm", bufs=7, name="rotp")
                        nc.tensor.matmul(rotp[:], rt_sb[:], qt[:], start=True, stop=True)
                        t1 = sb.tile([128, 512], BF16, tag="tmp", bufs=12, name="t1")
                        nc.vector.tensor_tensor(
                            t1[:], qt[:], cosf[:, 512 * c:512 * (c + 1)], ALU.mult)
                        t2 = sb.tile([128, 512], BF16, tag="tmp", bufs=12, name="t2")
                        nc.vector.tensor_tensor(
                            t2[:], rotp[:], sinf[:, 512 * c:512 * (c + 1)], ALU.mult)
                        q2 = sb.tile([128, 512], BF16, tag="tmp", bufs=12, name="q2")
                        qe = sb.tile([128, 512], BF16, tag="tmp", bufs=12, name="qe")
                        qr = sb.tile([128, 512], BF16, tag="tmp", bufs=12, name="qr")
                        if GPS:
                            nc.gpsimd.tensor_tensor(q2[:], t1[:], t2[:], ALU.add)
                        else:
                            nc.vector.tensor_tensor(q2[:], t1[:], t2[:], ALU.add)
                        nc.scalar.activation(qe[:], q2[:], AF.Exp)
                        nc.scalar.activation(qr[:], q2[:], AF.Relu)
                        # elu+1 = min(exp,1) + relu
                        nc.vector.scalar_tensor_tensor(
                            qf[j][:, 512 * c:512 * (c + 1)], qe[:], 1.0, qr[:],
                            ALU.min, ALU.add)

            # ---- K phase (token-major) ----
            wk_sb = sb.tile([128, NK, DQ], BF16, tag="w", bufs=3)
            nc.sync.dma_start(wk_sb[:], wk.rearrange("(k p) c -> p k c", p=128))
            if with_bk:
                bk_sb = sb.tile([1, DQ], BF16, tag="brow", bufs=3)
                nc.sync.dma_start(bk_sb[:], bk[:])
            cost = sb.tile([128, NT, DQ], BF16, tag="tm", bufs=2)
            nc.sync.dma_start(cost[:], costm.rearrange("p (m c) -> p m c", m=NT))
            sint = sb.tile([128, NT, DQ], BF16, tag="tm", bufs=2)
            nc.sync.dma_start(sint[:], sintm.rearrange("p (m c) -> p m c", m=NT))
            xkt = []
            for k in range(NK):
                t_ = sb.tile([128, T], BF16, tag="xt", bufs=12, name=f"xkt{k}")
                nc.sync.dma_start(t_[:], xkt_d[128 * k:128 * (k + 1), :])
                xkt.append(t_)

            kf = [sb.tile([128, DQ], BF16, tag="kf", bufs=NT, name=f"kf{m}")
                  for m in range(NT)]

            with nc.named_scope("kproj"):
                for m in range(NT):
                    psk = ps.tile([128, 512], F32, tag="mm", bufs=7, name="psk")
                    first = True
                    if with_bk:
                        nc.tensor.matmul(psk[:], ones[:, 0:128], bk_sb[:],
                                         start=True, stop=False)
                        first = False
                    for k in range(NK):
                        nc.tensor.matmul(
                            psk[:], xkt[k][:, 128 * m:128 * (m + 1)],
                            wk_sb[:, k, :], start=first, stop=(k == NK - 1))
                        first = False
                    ksb = sb.tile([128, 512], BF16, tag="tmp", bufs=12, name="ksb")
                    nc.scalar.copy(ksb[:], psk[:])
                    # RoPE token-major, [evens|odds] per-head halves
                    kv8 = ksb.rearrange("p (h s i) -> p h s i", h=NH, s=2, i=32)
                    t1 = sb.tile([128, 512], BF16, tag="tmp", bufs=12, name="t1k")
                    nc.vector.tensor_tensor(t1[:], ksb[:], cost[:, m, :], ALU.mult)
                    t2 = sb.tile([128, 512], BF16, tag="tmp", bufs=12, name="t2k")
                    t28 = t2.rearrange("p (h s i) -> p h s i", h=NH, s=2, i=32)
                    sin8 = sint[:, m, :].rearrange("p (h s i) -> p h s i", h=NH, s=2, i=32)
                    nc.vector.tensor_tensor(t28[:, :, 0, :], kv8[:, :, 1, :],
                                            sin8[:, :, 0, :], ALU.mult)
                    nc.vector.tensor_tensor(t28[:, :, 1, :], kv8[:, :, 0, :],
                                            sin8[:, :, 1, :], ALU.mult)
                    k2 = sb.tile([128, 512], BF16, tag="tmp", bufs=12, name="k2")
                    ke = sb.tile([128, 512], BF16, tag="tmp", bufs=12, name="ke")
                    kr = sb.tile([128, 512], BF16, tag="tmp", bufs=12, name="kr")
                    if GPS:
                        nc.gpsimd.tensor_tensor(k2[:], t1[:], t2[:], ALU.add)
                    else:
                        nc.vector.tensor_tensor(k2[:], t1[:], t2[:], ALU.add)
                    nc.scalar.activation(ke[:], k2[:], AF.Exp)
                    nc.scalar.activation(kr[:], k2[:], AF.Relu)
                    nc.vector.scalar_tensor_tensor(kf[m][:], ke[:], 1.0, kr[:],
                                                   ALU.min, ALU.add)

            # ---- V phase + kv accumulation ----
            wv_sb = sb.tile([128, NK, DQ], BF16, tag="w", bufs=3)
            nc.sync.dma_start(wv_sb[:], wv.rearrange("(k p) c -> p k c", p=128))
            if with_bv:
                bv_sb = sb.tile([1, DQ], BF16, tag="brow", bufs=3)
                nc.sync.dma_start(bv_sb[:], bv[:])
            mk_sb = sb.tile([128, NT], F32, tag="mask")
            nc.sync.dma_start(mk_sb[:], maskc[:])
            wo_sb = sb.tile([128, NJ, D_MODEL], BF16, tag="wo")
            nc.sync.dma_start(wo_sb[:], wo.rearrange("(k p) c -> p k c", p=128))
            xvt = []
            for k in range(NK):
                t_ = sb.tile([128, T], BF16, tag="xt", bufs=12, name=f"xvt{k}")
                nc.sync.dma_start(t_[:], xvt_d[128 * k:128 * (k + 1), :])
                xvt.append(t_)

            kvp_t = ps.tile([128, 512], F32, tag="kv", bufs=1, name="kvp")
            kvp = kvp_t[:, 0:272]
            # open the kv accumulation group: zero the whole region so later
            # disjoint-region matmuls (start=False) all accumulate onto it
            nc.tensor.matmul(kvp[:], zrow[:, 0:128], zrow[:, 0:272],
                             start=True, stop=False)
            with nc.named_scope("vproj"):
                for m in range(NT):
                    psv = ps.tile([128, 512], F32, tag="mm", bufs=7, name="psv")
                    first = True
                    if with_bv:
                        nc.tensor.matmul(psv[:], ones[:, 0:128], bv_sb[:],
                                         start=True, stop=False)
                        first = False
                    for k in range(NK):
                        nc.tensor.matmul(
                            psv[:], xvt[k][:, 128 * m:128 * (m + 1)],
                            wv_sb[:, k, :], start=first, stop=(k == NK - 1))
                        first = False
                    v2 = sb.tile([128, NH, 68], BF16, tag="vv", bufs=4, name="v2")
                    nc.vector.tensor_scalar_mul(
                        v2[:, :, 0:64], psv.rearrange("p (h i) -> p h i", h=NH),
                        mk_sb[:, m:m + 1])
                    nc.vector.tensor_copy(
                        v2[:, :, 64:68],
                        mk_sb[:, m:m + 1].rearrange("p (a i) -> p a i", a=1)
                        .broadcast_to([128, NH, 4]))
                    # kv' accumulation: head h -> rows 64*(h%2), cols 68*(h//2).
                    for h in range(NH):
                        r0 = 64 * (h % 2)
                        c0 = 68 * (h // 2)
                        nc.tensor.matmul(
                            kvp[r0:r0 + 64, c0:c0 + 68],
                            kf[m][:, 64 * h:64 * (h + 1)],
                            v2[:, h, :],
                            start=False, stop=False,
                            tile_position=(0, r0))
            # close the kv group (single dep covering all kv matmuls)
            nc.tensor.matmul(kvp[:], zrow[:, 0:128], zrow[:, 0:272],
                             start=False, stop=True)

            # repack kv' into per-j block-diagonal [128,128] + ksum pack [128,8]
            kvblk = [sb.tile([128, 128], BF16, tag="kvb", bufs=NJ, name=f"kvb{j}")
                     for j in range(NJ)]
            kspack = sb.tile([128, 8], BF16, tag="ksp")
            nc.vector.memset(kspack[:], 0.0)
            for j in range(NJ):
                nc.vector.memset(kvblk[j][:], 0.0)
                nc.vector.tensor_copy(kvblk[j][0:64, 0:64],
                                      kvp[0:64, 68 * j:68 * j + 64])
                nc.vector.tensor_copy(kvblk[j][64:128, 64:128],
                                      kvp[64:128, 68 * j:68 * j + 64])
                nc.vector.tensor_copy(kspack[0:64, 2 * j:2 * j + 1],
                                      kvp[0:64, 68 * j + 64:68 * j + 65])
                nc.vector.tensor_copy(kspack[64:128, 2 * j + 1:2 * j + 2],
                                      kvp[64:128, 68 * j + 64:68 * j + 65])

            # esel: replicated per-stripe broadcast selectors for zexp
            esel = sb.tile([128, 128], BF16, tag="esel")
            nc.sync.dma_start(esel[:], eselp[:])

            # ---- attention (feature-major) ----
            osb = [sb.tile([128, T], BF16, tag="osb", bufs=NJ, name=f"osb{j}")
                   for j in range(NJ)]
            with nc.named_scope("attn"):
                for c in range(NC_):
                    ch = slice(512 * c, 512 * (c + 1))
                    den = ps.tile([128, 512], F32, tag="mm", bufs=7, name="den")
                    for j in range(NJ):
                        nc.tensor.matmul(den[32 * j:32 * j + 2, :],
                                         kspack[:, 2 * j:2 * j + 2],
                                         qf[j][:, ch], start=True, stop=True,
                                         tile_position=(0, 32 * j))
                    zrec = sb.tile([128, 512], BF16, tag="zr", bufs=2, name="zrec")
                    with nc.allow_low_precision(reason="z scale in bf16 is fine"):
                        nc.vector.reciprocal(zrec[:], den[:])
                    for j in range(NJ):
                        zep = ps.tile([128, 512], F32, tag="mm", bufs=7, name="zep")
                        nc.tensor.matmul(zep[:], esel[32 * j:32 * j + 2, :],
                                         zrec[32 * j:32 * j + 2, :],
                                         start=True, stop=True,
                                         tile_position=(32 * j, 0))
                        zes = sb.tile([128, 512], BF16, tag="ze", bufs=4, name="zes")
                        nc.scalar.copy(zes[:], zep[:])
                        opp = ps.tile([128, 512], F32, tag="mm", bufs=7, name="opp")
                        nc.tensor.matmul(opp[:], kvblk[j][:], qf[j][:, ch],
                                         start=True, stop=True)
                        if GPS and j >= 2:
                            ops = sb.tile([128, 512], BF16, tag="tmp", bufs=12,
                                          name="ops")
                            nc.scalar.copy(ops[:], opp[:])
                            nc.gpsimd.tensor_tensor(osb[j][:, ch], ops[:], zes[:],
                                                    ALU.mult)
                        else:
                            nc.vector.tensor_tensor(osb[j][:, ch], opp[:], zes[:],
                                                    ALU.mult)

            # ---- output projection ----
            with nc.named_scope("yproj"):
                for m in range(NT):
                    yps = []
                    for c2 in range(2):
                        yps.append(ps.tile([128, 512], F32, tag="mm", bufs=7,
                                           name="yp"))
                    for j in range(NJ):
                        for c2 in range(2):
                            nc.tensor.matmul(
                                yps[c2][:], osb[j][:, 128 * m:128 * (m + 1)],
                                wo_sb[:, j, 512 * c2:512 * (c2 + 1)],
                                start=(j == 0), stop=(j == NJ - 1))
                    for c2 in range(2):
                        ysb = sb.tile([128, 512], F32, tag="ysb", bufs=4, name="ysb")
                        if c2 == 0:
                            nc.scalar.copy(ysb[:], yps[c2][:])
                        else:
                            nc.vector.tensor_copy(ysb[:], yps[c2][:])
                        nc.scalar.dma_start(
                            y[128 * m:128 * (m + 1), 512 * c2:512 * (c2 + 1)],
                            ysb[:])

    nc.compile()
    return nc


def _host_prep(queries, keys, values, key_lengths, Wq, bq, Wk, bk, Wv, bv, Wo):
    """Build the per-core input maps (host side: transpose, cast, tables)."""
    B = queries.shape[0]
    # per-head [evens|odds] feature permutation
    pat = np.concatenate([np.arange(0, HEAD_DIM, 2), np.arange(1, HEAD_DIM, 2)])
    perm = np.concatenate([h * HEAD_DIM + pat for h in range(NH)])  # within 512

    inv_freq = 1.0 / (ROPE_THETA ** (np.arange(0, HEAD_DIM, 2, dtype=np.float64)
                                     / HEAD_DIM))  # [32]
    t = np.arange(T, dtype=np.float64)
    ang = t[:, None] * inv_freq[None, :]           # [T, 32]
    cos32 = np.cos(ang).astype(np.float32)
    sin32 = np.sin(ang).astype(np.float32)

    # feature-major tables [128, T]: row r: block = r % 64; i = block % 32
    idx = np.arange(128) % HEAD_DIM
    fidx = np.where(idx < 32, idx, idx - 32)
    cosfm = cos32[:, fidx].T.astype(BF)            # [128, T]
    sinfm = sin32[:, fidx].T.astype(BF)

    # token-major tables [T, 512] -> [128, NT*512] partition-major
    cidx = np.arange(DQ) % HEAD_DIM
    cf = np.where(cidx < 32, cidx, cidx - 32)
    costm_full = cos32[:, cf]                      # [T, 512]
    ssign = np.where(cidx < 32, -1.0, 1.0).astype(np.float32)
    sintm_full = sin32[:, cf] * ssign
    def tmshape(a):  # [T, 512] -> [128, NT*512]
        return np.ascontiguousarray(
            a.reshape(NT, 128, DQ).transpose(1, 0, 2).reshape(128, NT * DQ)).astype(BF)
    costm = tmshape(costm_full)
    sintm = tmshape(sintm_full)

    # rotation matrix R (permuted layout), lhsT = R.T
    R = np.zeros((128, 128), np.float32)
    for base in (0, 64):
        R[base + 0:base + 32, base + 32:base + 64] = -np.eye(32)
        R[base + 32:base + 64, base + 0:base + 32] = np.eye(32)
    rt = np.ascontiguousarray(R.T).astype(BF)
    esel = np.zeros((128, 128), np.float32)
    for j in range(NJ):
        esel[32 * j, 0:64] = 1.0
        esel[32 * j + 1, 64:128] = 1.0
    esel = esel.astype(BF)

    with_bq = bool(np.any(np.asarray(bq)))
    with_bk = bool(np.any(np.asarray(bk)))
    with_bv = bool(np.any(np.asarray(bv)))

    xqt_b = [np.ascontiguousarray(np.asarray(queries[b]).astype(BF).T)
             for b in range(B)]
    xkt_b = [np.ascontiguousarray(np.asarray(keys[b]).astype(BF).T)
             for b in range(B)]
    xvt_b = [np.ascontiguousarray(np.asarray(values[b]).astype(BF).T)
             for b in range(B)]
    Wq = np.asarray(Wq, np.float32); Wk = np.asarray(Wk, np.float32)
    Wv = np.asarray(Wv, np.float32); Wo = np.asarray(Wo, np.float32)
    bq = np.asarray(bq, np.float32); bk = np.asarray(bk, np.float32)
    bv = np.asarray(bv, np.float32)

    in_maps = []
    for core in range(2 * B):
        b, g = core // 2, core % 2
        sl = slice(DQ * g, DQ * (g + 1))
        mask = (np.arange(T) < int(key_lengths[b])).astype(np.float32)
        maskc = np.ascontiguousarray(mask.reshape(NT, 128).T)
        m = {
            "xqt": xqt_b[b], "xkt": xkt_b[b], "xvt": xvt_b[b],
            "wq": np.ascontiguousarray(Wq[:, sl][:, perm]).astype(BF),
            "wk": np.ascontiguousarray(Wk[:, sl][:, perm]).astype(BF),
            "wv": np.ascontiguousarray(Wv[:, sl]).astype(BF),
            "wo": np.ascontiguousarray(Wo[sl, :]).astype(BF),
            "cosfm": cosfm, "sinfm": sinfm, "costm": costm, "sintm": sintm,
            "rt": rt, "esel": esel, "maskc": maskc,
        }
        if with_bq:
            m["bq"] = bq[sl][perm].reshape(1, DQ).astype(BF)
        if with_bk:
            m["bk"] = bk[sl][perm].reshape(1, DQ).astype(BF)
        if with_bv:
            m["bv"] = bv[sl].reshape(1, DQ).astype(BF)
        in_maps.append(m)
    return in_maps, (with_bq, with_bk, with_bv)


def kernel(queries, keys, values, attn_mask, query_lengths, key_lengths,
           Wq, bq, Wk, bk, Wv, bv, Wo, bo):
    global LAST_RESULTS
    B = queries.shape[0]
    in_maps, bias_flags = _host_prep(queries, keys, values, key_lengths,
                                     Wq, bq, Wk, bk, Wv, bv, Wo)
    nc = _build_program(*bias_flags)
    res = run_bass_kernel_spmd(nc, in_maps, core_ids=list(range(2 * B)))
    LAST_RESULTS = res
    bo = np.asarray(bo, np.float32)
    out = np.zeros((B, T, D_MODEL), np.float32)
    for b in range(B):
        if int(key_lengths[b]) == 0:
            # kv/ksum are all-zero; reference output is exactly bo
            out[b] = bo[None, :]
        else:
            out[b] = res.results[2 * b]["y"] + res.results[2 * b + 1]["y"] + bo
    return out
